# revision 19
# baseline (speedup 1.0000x reference)
"""DINOv2 self-attention (QKV projection + SDPA, no out-proj) on 8 Trainium2
NeuronCores.

Sharding: pure data-parallel over batch (B=8 -> one batch element per core);
no cross-core communication.

Host-side prep inside kernel(): transpose hidden_states to x.T per batch and
pack W as W.T = [Wq.T | Wk.T | Wv.T], so every on-chip matmul operand already
has its contraction dim on the partition axis.

Per-core program (S=1370, D=1024, H=16, hd=64), all matmuls in float32r:
  Phase A: v = x @ Wv^T + bv scattered into v_ext with a ones-column per head
     (v_ext[:, t, h*65+64] = 1) so the softmax denominator falls out of the
     ctx matmul as a 65th output row.  Then q/k projection for head-pair 0.
  Phase B (head-pair pipeline): for hp in 0..7, attention for heads
     2hp,2hp+1 over all of S (sq chunks of <=512), while the q/k projections
     for head-pair hp+1 are drip-fed into the same PE instruction stream
     (2 matmuls per kt block) and the previous chunk's ctx.T finalization
     (PE transpose + DVE normalize, 1 item per kt block) rides along too.
     This keeps ACT's exp stream (the second-busiest engine) running from
     ~45us onward instead of idling during a monolithic projection phase.
  scoresT[sk, sq] = kT^T @ qT per head at partition offsets 0/64 (row-group
     pairs), exp via ACT with fused 1/8 scale (max-subtraction skipped:
     |scores/8| <= ~8.7 fits fp32 easily), ctx.T[65, sq] accumulated over the
     11 sk tiles with [v | 1] stationary.
  Flush: ctx.T transposed back per 128-wide sub-tile via PE transpose, then
     out[:, hp*128:+128] = ctx * (1/denom) on DVE, column-block DMA to out.

PSUM budget (8 banks): scores double-buffer 2x[128,2,512] = 4, ctx
accumulator [65,2,512] = 2, projection pass [128,512] = 1, transpose
scratch [128,65] = 1.
"""

import numpy as np
from contextlib import ExitStack

import concourse.bass as bass
import concourse.bacc as bacc
import concourse.tile as tile
from concourse import mybir
from concourse import bass_utils
from concourse.masks import make_identity

S, D, H, HD = 1370, 1024, 16, 64
F32 = mybir.dt.float32
F32R = mybir.dt.float32r
BF16 = mybir.dt.bfloat16
ND = D // 128                      # 8 contraction tiles
NT = (S + 127) // 128              # 11 token tiles
TSZ = [min(128, S - i * 128) for i in range(NT)]
CHUNKS = [(0, 512), (512, 512), (1024, S - 1024)]
EXP = mybir.ActivationFunctionType.Exp
NHP = 8                            # head pairs
# ablation knobs (diagnostics only; breaks correctness when not default)
ABL_NO_NORM = False     # skip reciprocal+mul (copy raw ctx.T)
ABL_NO_FLUSH = False    # skip transposes too; DMA cst directly
ABL_NO_CTX = False      # skip ctx matmuls (and flush is fed garbage psc)
ABL_NO_EXP = False      # skip exp (et stays garbage)
ABL_NO_SCORES = False   # scores only for kt==0
ABL_NO_OUT = False      # skip output DMAs
EXP_SPLIT = 1           # exp instructions per (hp, kt): 1 or 2


class _Pump:
    """Wraps a generator; pump(n) advances up to n yields, returns number
    actually advanced (0 once exhausted)."""

    def __init__(self, gen):
        self.gen = gen
        self.done = gen is None

    def pump(self, n):
        if self.done:
            return 0
        c = 0
        try:
            while c < n:
                next(self.gen)
                c += 1
        except StopIteration:
            self.done = True
        return c

    def drain(self):
        while not self.done:
            self.pump(64)


def _one_pass(tc, P, xT, wqk2, wv2, out):
    nc = tc.nc
    ident, bT_sb = P.ident, P.bT_sb
    xt = P.xt_pool.tile([128, ND, S], F32R, tag="xt", name="xt")
    v_ext = P.vext_pool.tile([128, NT, H * 65], F32R, tag="vext", name="v_ext")

    # ones columns (h*65+64) for the fused softmax denominator; memset
    # can't produce f32r, so synthesize 1.0 as in0*0 + 1 on DVE
    for t in range(NT):
        ones_view = v_ext[:, t, :].rearrange("p (h e) -> p h e", e=65)[:, :, 64]
        nc.vector.tensor_scalar(
            ones_view, bT_sb[:, 0:16],
            0.0, 1.0, mybir.AluOpType.mult, mybir.AluOpType.add)

    qk_tiles = {}

    def proj_gen(hpn):
        # q/k projection for head-pair hpn: 2 proj x 3 chunks x 8 d-tile
        # matmuls into a single psqk bank, bias added on DVE into qT/kT.
        # All 16 weight tiles arrive in ONE host-packed DMA (DVE queue).
        qTn = P.qk_pool.tile([128, S], F32R, tag="qT", name="qT")
        kTn = P.qk_pool.tile([128, S], F32R, tag="kT", name="kT")
        qk_tiles[hpn] = (qTn, kTn)
        w = P.wqk_pool.tile([128, 16, 128], F32R, tag="wqk", name="wqk")
        nc.sync.dma_start(w[:], wqk2[hpn])
        for proj in (1, 0):
            dstT = kTn if proj == 1 else qTn
            for (c0, cw) in CHUNKS:
                ps = P.psqk.tile([128, 512], F32, tag="psqk", name="psqk")
                for d in range(ND):
                    nc.tensor.matmul(
                        ps[:, :cw], w[:, proj * 8 + d, :], xt[:, d, c0:c0 + cw],
                        start=(d == 0), stop=(d == ND - 1))
                    yield
                nc.vector.tensor_scalar_add(
                    dstT[:, c0:c0 + cw], ps[:, :cw],
                    bT_sb[:, proj * 8 + hpn:proj * 8 + hpn + 1])
                yield

    wv_pool = P.wv_pool
    bvb_sb = wv_pool.tile([128, D], F32, tag="bvb", name="bvb_sb")

    def v_gen(half):
        # v projection for heads half*8..half*8+7, scattered into v_ext;
        # shares the psqk PSUM bank with the q/k projection stream.
        wv = wv_pool.tile([128, ND, 512], F32R, tag="wv", name="wv")
        nc.scalar.dma_start(wv[:], wv2[half])
        for t in range(NT):
            tsz = TSZ[t]
            ps = P.psqk.tile([128, 512], F32, tag="psqk", name="psv")
            for d in range(ND):
                nc.tensor.matmul(
                    ps[:tsz, :], xt[:, d, t * 128:t * 128 + tsz],
                    wv[:, d, :], start=(d == 0), stop=(d == ND - 1))
                yield
            dst = v_ext[:tsz, t, :].rearrange(
                "p (h e) -> p h e", e=65)[:, half * 8:(half + 1) * 8, 0:64]
            src = ps[:tsz, :].rearrange("p (h e) -> p h e", e=64)
            bias = bvb_sb[:tsz, half * 512:(half + 1) * 512].rearrange(
                "p (h e) -> p h e", e=64)
            nc.vector.tensor_add(dst, src, bias)
            yield

    # ---- phase A: x loads, v half 0, q/k projection for hp 0 ----
    for d in range(ND):
        nc.sync.dma_start(xt[:, d, :], xT[d * 128:(d + 1) * 128, :])
    nc.scalar.dma_start(bvb_sb[:], P.bvb[:])
    _Pump(v_gen(0)).drain()
    _Pump(proj_gen(0)).drain()
    vrest = _Pump(v_gen(1))

    # ---- phase B: per-head-pair attention with pipelined projections ----
    with ExitStack() as sB:
        pss = sB.enter_context(tc.tile_pool(name="pss", bufs=2, space="PSUM"))
        psc = sB.enter_context(tc.tile_pool(name="psc", bufs=1, space="PSUM"))
        tpp = sB.enter_context(tc.tile_pool(name="tpp", bufs=1, space="PSUM"))
        et_pool = sB.enter_context(tc.tile_pool(name="et", bufs=4))
        cs_pool = sB.enter_context(tc.tile_pool(name="cs", bufs=2))
        outp = sB.enter_context(tc.tile_pool(name="outp", bufs=3))
        rec_pool = sB.enter_context(tc.tile_pool(name="rec", bufs=4))

        def flush_gen(cst, hp, c0, cw):
            # finalize a completed (hp, chunk): per sub-tile PE transpose of
            # ctx.T, normalize by the denominator row on DVE, then ONE
            # strided DMA for the whole 128-wide output column block.
            subs = [(s0, min(128, cw - s0)) for s0 in range(0, cw, 128)]
            ot = outp.tile([128, 4, 128], F32, tag="out", name="out_sb")
            if ABL_NO_FLUSH:
                for si, (s0, ssz) in enumerate(subs):
                    nc.vector.tensor_copy(ot[:65, si, :], cst[:65, 0, s0:s0 + 128])
                    yield
            else:
              for si, (s0, ssz) in enumerate(subs):
                for hi in range(2):
                    tp = tpp.tile([128, 65], F32, tag="tp", name="tp")
                    nc.tensor.transpose(
                        tp[:ssz, :], cst[:65, hi, s0:s0 + ssz], ident[:65, :65])
                    if ABL_NO_NORM:
                        nc.vector.tensor_copy(
                            ot[:ssz, si, hi * 64:(hi + 1) * 64], tp[:ssz, 0:64])
                        yield
                        continue
                    rec = rec_pool.tile([128, 1], F32, tag="rec", name="rec")
                    nc.vector.reciprocal(rec[:ssz], tp[:ssz, 64:65])
                    nc.vector.tensor_scalar_mul(
                        ot[:ssz, si, hi * 64:(hi + 1) * 64], tp[:ssz, 0:64],
                        rec[:ssz])
                    yield
            if ABL_NO_OUT:
                return
            full, rem = cw // 128, cw % 128
            cols = slice(hp * 128, (hp + 1) * 128)
            nc.sync.dma_start(
                out[c0:c0 + full * 128, cols].rearrange(
                    "(sub p) c -> p sub c", p=128),
                ot[:, 0:full, :])
            if rem:
                nc.sync.dma_start(out[c0 + full * 128:c0 + cw, cols],
                                  ot[:rem, full, :])

        flush = _Pump(None)
        proj = _Pump(None)
        # v half 1 (heads 8-15) drips into hp 0-1 attention; needed by hp 4
        for hp in range(NHP):
            qT, kT = qk_tiles[hp]
            if hp + 1 < NHP:
                proj = _Pump(proj_gen(hp + 1))
            for (c0, cw) in CHUNKS:
                psc_t = psc.tile([65, 2, 512], F32, tag="psc", name="psc")
                ets = {}

                def emit_ctx(kt):
                    ksz = TSZ[kt]
                    et = ets.pop(kt)
                    if ABL_NO_CTX and kt not in (0, NT - 1):
                        return
                    for hi in range(2):
                        h = 2 * hp + hi
                        nc.tensor.matmul(
                            psc_t[:, hi, :cw],
                            v_ext[:ksz, kt, h * 65:(h + 1) * 65],
                            et[:ksz, hi, :cw],
                            start=(kt == 0), stop=(kt == NT - 1))

                for kt in range(NT):
                    ksz, k0 = TSZ[kt], kt * 128
                    ps_s = pss.tile([128, 2, 512], F32, tag="pss", name="pss")
                    et = et_pool.tile([128, 2, 512], F32R, tag="et", name="et")
                    ets[kt] = et
                    if not (ABL_NO_SCORES and kt > 0):
                        for hi in range(2):
                            p0 = hi * 64
                            nc.tensor.matmul(
                                ps_s[:ksz, hi, :cw],
                                kT[p0:p0 + 64, k0:k0 + ksz],
                                qT[p0:p0 + 64, c0:c0 + cw],
                                start=True, stop=True)
                    if not ABL_NO_EXP:
                        if EXP_SPLIT == 1:
                            nc.scalar.activation(
                                et[:ksz, :, :cw], ps_s[:ksz, :, :cw], EXP,
                                scale=0.125)
                        else:
                            h2 = cw // 2
                            nc.scalar.activation(
                                et[:ksz, :, 0:h2], ps_s[:ksz, :, 0:h2], EXP,
                                scale=0.125)
                            nc.scalar.activation(
                                et[:ksz, :, h2:cw], ps_s[:ksz, :, h2:cw], EXP,
                                scale=0.125)
                    else:
                        nc.vector.tensor_copy(
                            et[:ksz, :, :cw], ps_s[:ksz, :, :cw])
                    if kt > 0:
                        emit_ctx(kt - 1)
                    flush.pump(1)
                    proj.pump(2)
                    vrest.pump(3)
                emit_ctx(NT - 1)
                flush.drain()
                cst = cs_pool.tile([65, 2, 512], F32, tag="cs", name="cs")
                nc.vector.tensor_copy(cst[:, :, :cw], psc_t[:, :, :cw])
                flush = _Pump(flush_gen(cst, hp, c0, cw))
        flush.drain()
        proj.drain()
        vrest.drain()


class _Pools:
    pass


def _body(tc, xT, wqk2, wv2, bT, bvb, out, reps=1):
    nc = tc.nc
    with ExitStack() as ctx:
        P = _Pools()
        const = ctx.enter_context(tc.tile_pool(name="const", bufs=1))
        P.ident = const.tile([65, 65], F32)
        make_identity(nc, P.ident)
        P.bT_sb = const.tile([128, 24], F32)
        nc.sync.dma_start(P.bT_sb[:], bT[:])
        P.bvb = bvb
        P.xt_pool = ctx.enter_context(tc.tile_pool(name="xt", bufs=1))
        P.wv_pool = ctx.enter_context(tc.tile_pool(name="wv", bufs=1))
        P.vext_pool = ctx.enter_context(tc.tile_pool(name="vext", bufs=1))
        P.qk_pool = ctx.enter_context(tc.tile_pool(name="qkT", bufs=2))
        P.wqk_pool = ctx.enter_context(tc.tile_pool(name="wqk", bufs=2))
        P.psqk = ctx.enter_context(
            tc.tile_pool(name="psqk", bufs=1, space="PSUM"))
        for _rep in range(reps):
            _one_pass(tc, P, xT, wqk2, wv2, out)


def build_program(reps=1):
    nc = bacc.Bacc("TRN2", target_bir_lowering=False, debug=False,
                   num_devices=8)
    xT = nc.dram_tensor("xT", [D, S], F32R, kind="ExternalInput").ap()
    wqk2 = nc.dram_tensor("wqk2", [NHP, 128, 16 * 128], F32R,
                          kind="ExternalInput").ap()
    wv2 = nc.dram_tensor("wv2", [2, 128, ND * 512], F32R,
                         kind="ExternalInput").ap()
    bT = nc.dram_tensor("bT", [128, 24], F32, kind="ExternalInput").ap()
    bvb = nc.dram_tensor("bvb", [128, D], F32, kind="ExternalInput").ap()
    out = nc.dram_tensor("out", [S, D], F32, kind="ExternalOutput").ap()
    with tile.TileContext(nc) as tc:
        _body(tc, xT, wqk2, wv2, bT, bvb, out, reps=reps)
    nc.compile()
    return nc


_PROGRAM = None


def _get_program():
    global _PROGRAM
    if _PROGRAM is None:
        _PROGRAM = build_program()
    return _PROGRAM


def _prep_inputs(hidden_states, Wq, bq, Wk, bk, Wv, bv):
    hs = np.asarray(hidden_states, dtype=np.float32)
    B = hs.shape[0]
    xT = np.ascontiguousarray(hs.transpose(0, 2, 1))
    wT = np.concatenate(
        [np.asarray(Wq, dtype=np.float32).T,
         np.asarray(Wk, dtype=np.float32).T,
         np.asarray(Wv, dtype=np.float32).T], axis=1)
    # wqk2[hp, p, (proj*8+d)*128 + c] = wT[d*128+p, proj*D + hp*128 + c]
    w4 = wT[:, :2 * D].reshape(ND, 128, 2, NHP, 128)  # [d, p, proj, hp, c]
    wqk2 = np.ascontiguousarray(
        w4.transpose(3, 1, 2, 0, 4).reshape(NHP, 128, 16 * 128))
    # wv2[half, p, d*512 + c] = wT[d*128+p, 2D + half*512 + c]
    wv4 = wT[:, 2 * D:].reshape(ND, 128, 2, 512)      # [d, p, half, c]
    wv2 = np.ascontiguousarray(
        wv4.transpose(2, 1, 0, 3).reshape(2, 128, ND * 512))
    b_all = np.concatenate([np.asarray(bq, dtype=np.float32),
                            np.asarray(bk, dtype=np.float32),
                            np.asarray(bv, dtype=np.float32)])
    bT_np = np.ascontiguousarray(b_all.reshape(24, 128).T)
    bvb_np = np.ascontiguousarray(
        np.broadcast_to(np.asarray(bv, dtype=np.float32), (128, D)))
    return [{"xT": xT[b], "wqk2": wqk2, "wv2": wv2, "bT": bT_np,
             "bvb": bvb_np} for b in range(B)]


def run(in_maps, **kw):
    nc = _get_program()
    return bass_utils.run_bass_kernel_spmd(
        nc, in_maps, core_ids=list(range(len(in_maps))), **kw)


def kernel(hidden_states, Wq, bq, Wk, bk, Wv, bv):
    in_maps = _prep_inputs(hidden_states, Wq, bq, Wk, bk, Wv, bv)
    res = run(in_maps)
    return np.stack([res.results[b]["out"] for b in range(len(in_maps))],
                    axis=0)


# revision 20
# speedup vs baseline: 1.0074x; 1.0074x over previous
"""DINOv2 self-attention (QKV projection + SDPA, no out-proj) on 8 Trainium2
NeuronCores.

Sharding: pure data-parallel over batch (B=8 -> one batch element per core);
no cross-core communication.

Host-side prep inside kernel(): transpose hidden_states to x.T per batch and
pack W as W.T = [Wq.T | Wk.T | Wv.T], so every on-chip matmul operand already
has its contraction dim on the partition axis.

Per-core program (S=1370, D=1024, H=16, hd=64), all matmuls in float32r:
  Phase A: v = x @ Wv^T + bv scattered into v_ext with a ones-column per head
     (v_ext[:, t, h*65+64] = 1) so the softmax denominator falls out of the
     ctx matmul as a 65th output row.  Then q/k projection for head-pair 0.
  Phase B (head-pair pipeline): for hp in 0..7, attention for heads
     2hp,2hp+1 over all of S (sq chunks of <=512), while the q/k projections
     for head-pair hp+1 are drip-fed into the same PE instruction stream
     (2 matmuls per kt block) and the previous chunk's ctx.T finalization
     (PE transpose + DVE normalize, 1 item per kt block) rides along too.
     This keeps ACT's exp stream (the second-busiest engine) running from
     ~45us onward instead of idling during a monolithic projection phase.
  scoresT[sk, sq] = kT^T @ qT per head at partition offsets 0/64 (row-group
     pairs), exp via ACT with fused 1/8 scale (max-subtraction skipped:
     |scores/8| <= ~8.7 fits fp32 easily), ctx.T[65, sq] accumulated over the
     11 sk tiles with [v | 1] stationary.
  Flush: ctx.T transposed back per 128-wide sub-tile via PE transpose, then
     out[:, hp*128:+128] = ctx * (1/denom) on DVE, column-block DMA to out.

PSUM budget (8 banks): scores double-buffer 2x[128,2,512] = 4, ctx
accumulator [65,2,512] = 2, projection pass [128,512] = 1, transpose
scratch [128,65] = 1.
"""

import numpy as np
from contextlib import ExitStack

import concourse.bass as bass
import concourse.bacc as bacc
import concourse.tile as tile
from concourse import mybir
from concourse import bass_utils
from concourse.masks import make_identity

S, D, H, HD = 1370, 1024, 16, 64
F32 = mybir.dt.float32
F32R = mybir.dt.float32r
BF16 = mybir.dt.bfloat16
ND = D // 128                      # 8 contraction tiles
NT = (S + 127) // 128              # 11 token tiles
TSZ = [min(128, S - i * 128) for i in range(NT)]
CHUNKS = [(0, 512), (512, 512), (1024, S - 1024)]
EXP = mybir.ActivationFunctionType.Exp
NHP = 8                            # head pairs
# ablation knobs (diagnostics only; breaks correctness when not default)
ABL_NO_NORM = False     # skip reciprocal+mul (copy raw ctx.T)
ABL_NO_FLUSH = False    # skip transposes too; DMA cst directly
ABL_NO_CTX = False      # skip ctx matmuls (and flush is fed garbage psc)
ABL_NO_EXP = False      # skip exp (et stays garbage)
ABL_NO_SCORES = False   # scores only for kt==0
ABL_NO_OUT = False      # skip output DMAs
EXP_SPLIT = 1           # exp instructions per (hp, kt): 1 or 2


class _Pump:
    """Wraps a generator; pump(n) advances up to n yields, returns number
    actually advanced (0 once exhausted)."""

    def __init__(self, gen):
        self.gen = gen
        self.done = gen is None

    def pump(self, n):
        if self.done:
            return 0
        c = 0
        try:
            while c < n:
                next(self.gen)
                c += 1
        except StopIteration:
            self.done = True
        return c

    def drain(self):
        while not self.done:
            self.pump(64)


def _one_pass(tc, P, xT, wqk2, wv2, out):
    nc = tc.nc
    ident, bT_sb = P.ident, P.bT_sb
    xt = P.xt_pool.tile([128, ND, S], F32R, tag="xt", name="xt")
    v_ext = P.vext_pool.tile([128, NT, H * 65], F32R, tag="vext", name="v_ext")

    # ones columns (h*65+64) for the fused softmax denominator; memset
    # can't produce f32r, so synthesize 1.0 as in0*0 + 1 on DVE
    for t in range(NT):
        ones_view = v_ext[:, t, :].rearrange("p (h e) -> p h e", e=65)[:, :, 64]
        nc.vector.tensor_scalar(
            ones_view, bT_sb[:, 0:16],
            0.0, 1.0, mybir.AluOpType.mult, mybir.AluOpType.add)

    qk_tiles = {}

    def proj_gen(hpn):
        # q/k projection for head-pair hpn: 2 proj x 3 chunks x 8 d-tile
        # matmuls into a single psqk bank, bias added on DVE into qT/kT.
        # All 16 weight tiles arrive in ONE host-packed DMA (DVE queue).
        qTn = P.qk_pool.tile([128, S], F32R, tag="qT", name="qT")
        kTn = P.qk_pool.tile([128, S], F32R, tag="kT", name="kT")
        qk_tiles[hpn] = (qTn, kTn)
        w = P.wqk_pool.tile([128, 16, 128], F32R, tag="wqk", name="wqk")
        nc.sync.dma_start(w[:], wqk2[hpn])
        for proj in (1, 0):
            dstT = kTn if proj == 1 else qTn
            for (c0, cw) in CHUNKS:
                ps = P.psqk.tile([128, 512], F32, tag="psqk", name="psqk")
                for d in range(ND):
                    nc.tensor.matmul(
                        ps[:, :cw], w[:, proj * 8 + d, :], xt[:, d, c0:c0 + cw],
                        start=(d == 0), stop=(d == ND - 1))
                    yield
                nc.vector.tensor_scalar_add(
                    dstT[:, c0:c0 + cw], ps[:, :cw],
                    bT_sb[:, proj * 8 + hpn:proj * 8 + hpn + 1])
                yield

    wv_pool = P.wv_pool
    bvb_sb = wv_pool.tile([128, D], F32, tag="bvb", name="bvb_sb")

    def v_gen(half):
        # v projection for heads half*8..half*8+7, scattered into v_ext;
        # shares the psqk PSUM bank with the q/k projection stream.
        wv = wv_pool.tile([128, ND, 512], F32R, tag="wv", name="wv")
        nc.scalar.dma_start(wv[:], wv2[half])
        for t in range(NT):
            tsz = TSZ[t]
            ps = P.psqk.tile([128, 512], F32, tag="psqk", name="psv")
            for d in range(ND):
                nc.tensor.matmul(
                    ps[:tsz, :], xt[:, d, t * 128:t * 128 + tsz],
                    wv[:, d, :], start=(d == 0), stop=(d == ND - 1))
                yield
            dst = v_ext[:tsz, t, :].rearrange(
                "p (h e) -> p h e", e=65)[:, half * 8:(half + 1) * 8, 0:64]
            src = ps[:tsz, :].rearrange("p (h e) -> p h e", e=64)
            bias = bvb_sb[:tsz, half * 512:(half + 1) * 512].rearrange(
                "p (h e) -> p h e", e=64)
            nc.vector.tensor_add(dst, src, bias)
            yield

    # ---- phase A: x loads, v half 0, q/k projection for hp 0 ----
    for d in range(ND):
        nc.sync.dma_start(xt[:, d, :], xT[d * 128:(d + 1) * 128, :])
    nc.scalar.dma_start(bvb_sb[:], P.bvb[:])
    _Pump(v_gen(0)).drain()
    _Pump(proj_gen(0)).drain()
    _Pump(v_gen(1)).drain()
    vrest = _Pump(None)

    # ---- phase B: per-head-pair attention with pipelined projections ----
    with ExitStack() as sB:
        pss = sB.enter_context(tc.tile_pool(name="pss", bufs=2, space="PSUM"))
        psc = sB.enter_context(tc.tile_pool(name="psc", bufs=1, space="PSUM"))
        tpp = sB.enter_context(tc.tile_pool(name="tpp", bufs=1, space="PSUM"))
        et_pool = sB.enter_context(tc.tile_pool(name="et", bufs=4))
        cs_pool = sB.enter_context(tc.tile_pool(name="cs", bufs=2))
        outp = sB.enter_context(tc.tile_pool(name="outp", bufs=3))
        rec_pool = sB.enter_context(tc.tile_pool(name="rec", bufs=4))

        def flush_gen(cst, hp, c0, cw):
            # finalize a completed (hp, chunk): per sub-tile PE transpose of
            # ctx.T, normalize by the denominator row on DVE, then ONE
            # strided DMA for the whole 128-wide output column block.
            subs = [(s0, min(128, cw - s0)) for s0 in range(0, cw, 128)]
            ot = outp.tile([128, 4, 128], F32, tag="out", name="out_sb")
            if ABL_NO_FLUSH:
                for si, (s0, ssz) in enumerate(subs):
                    nc.vector.tensor_copy(ot[:65, si, :], cst[:65, 0, s0:s0 + 128])
                    yield
            else:
              for si, (s0, ssz) in enumerate(subs):
                for hi in range(2):
                    tp = tpp.tile([128, 65], F32, tag="tp", name="tp")
                    nc.tensor.transpose(
                        tp[:ssz, :], cst[:65, hi, s0:s0 + ssz], ident[:65, :65])
                    if ABL_NO_NORM:
                        nc.vector.tensor_copy(
                            ot[:ssz, si, hi * 64:(hi + 1) * 64], tp[:ssz, 0:64])
                        yield
                        continue
                    rec = rec_pool.tile([128, 1], F32, tag="rec", name="rec")
                    nc.vector.reciprocal(rec[:ssz], tp[:ssz, 64:65])
                    nc.vector.tensor_scalar_mul(
                        ot[:ssz, si, hi * 64:(hi + 1) * 64], tp[:ssz, 0:64],
                        rec[:ssz])
                    yield
            if ABL_NO_OUT:
                return
            full, rem = cw // 128, cw % 128
            cols = slice(hp * 128, (hp + 1) * 128)
            nc.sync.dma_start(
                out[c0:c0 + full * 128, cols].rearrange(
                    "(sub p) c -> p sub c", p=128),
                ot[:, 0:full, :])
            if rem:
                nc.sync.dma_start(out[c0 + full * 128:c0 + cw, cols],
                                  ot[:rem, full, :])

        flush = _Pump(None)
        proj = _Pump(None)
        # v half 1 (heads 8-15) drips into hp 0-1 attention; needed by hp 4
        for hp in range(NHP):
            qT, kT = qk_tiles[hp]
            if hp + 1 < NHP:
                proj = _Pump(proj_gen(hp + 1))
            for (c0, cw) in CHUNKS:
                psc_t = psc.tile([65, 2, 512], F32, tag="psc", name="psc")
                ets = {}

                def emit_ctx(kt):
                    ksz = TSZ[kt]
                    et = ets.pop(kt)
                    if ABL_NO_CTX and kt not in (0, NT - 1):
                        return
                    for hi in range(2):
                        h = 2 * hp + hi
                        nc.tensor.matmul(
                            psc_t[:, hi, :cw],
                            v_ext[:ksz, kt, h * 65:(h + 1) * 65],
                            et[:ksz, hi, :cw],
                            start=(kt == 0), stop=(kt == NT - 1))

                for kt in range(NT):
                    ksz, k0 = TSZ[kt], kt * 128
                    ps_s = pss.tile([128, 2, 512], F32, tag="pss", name="pss")
                    et = et_pool.tile([128, 2, 512], F32R, tag="et", name="et")
                    ets[kt] = et
                    if not (ABL_NO_SCORES and kt > 0):
                        for hi in range(2):
                            p0 = hi * 64
                            nc.tensor.matmul(
                                ps_s[:ksz, hi, :cw],
                                kT[p0:p0 + 64, k0:k0 + ksz],
                                qT[p0:p0 + 64, c0:c0 + cw],
                                start=True, stop=True)
                    if not ABL_NO_EXP:
                        if EXP_SPLIT == 1:
                            nc.scalar.activation(
                                et[:ksz, :, :cw], ps_s[:ksz, :, :cw], EXP,
                                scale=0.125)
                        else:
                            h2 = cw // 2
                            nc.scalar.activation(
                                et[:ksz, :, 0:h2], ps_s[:ksz, :, 0:h2], EXP,
                                scale=0.125)
                            nc.scalar.activation(
                                et[:ksz, :, h2:cw], ps_s[:ksz, :, h2:cw], EXP,
                                scale=0.125)
                    else:
                        nc.vector.tensor_copy(
                            et[:ksz, :, :cw], ps_s[:ksz, :, :cw])
                    if kt > 0:
                        emit_ctx(kt - 1)
                    flush.pump(1)
                    proj.pump(2 if kt % 3 else 1)
                    vrest.pump(0)
                emit_ctx(NT - 1)
                flush.drain()
                cst = cs_pool.tile([65, 2, 512], F32, tag="cs", name="cs")
                nc.vector.tensor_copy(cst[:, :, :cw], psc_t[:, :, :cw])
                flush = _Pump(flush_gen(cst, hp, c0, cw))
        flush.drain()
        proj.drain()
        vrest.drain()


class _Pools:
    pass


def _body(tc, xT, wqk2, wv2, bT, bvb, out, reps=1):
    nc = tc.nc
    with ExitStack() as ctx:
        P = _Pools()
        const = ctx.enter_context(tc.tile_pool(name="const", bufs=1))
        P.ident = const.tile([65, 65], F32)
        make_identity(nc, P.ident)
        P.bT_sb = const.tile([128, 24], F32)
        nc.sync.dma_start(P.bT_sb[:], bT[:])
        P.bvb = bvb
        P.xt_pool = ctx.enter_context(tc.tile_pool(name="xt", bufs=1))
        P.wv_pool = ctx.enter_context(tc.tile_pool(name="wv", bufs=1))
        P.vext_pool = ctx.enter_context(tc.tile_pool(name="vext", bufs=1))
        P.qk_pool = ctx.enter_context(tc.tile_pool(name="qkT", bufs=2))
        P.wqk_pool = ctx.enter_context(tc.tile_pool(name="wqk", bufs=2))
        P.psqk = ctx.enter_context(
            tc.tile_pool(name="psqk", bufs=1, space="PSUM"))
        for _rep in range(reps):
            _one_pass(tc, P, xT, wqk2, wv2, out)


def build_program(reps=1):
    nc = bacc.Bacc("TRN2", target_bir_lowering=False, debug=False,
                   num_devices=8)
    xT = nc.dram_tensor("xT", [D, S], F32R, kind="ExternalInput").ap()
    wqk2 = nc.dram_tensor("wqk2", [NHP, 128, 16 * 128], F32R,
                          kind="ExternalInput").ap()
    wv2 = nc.dram_tensor("wv2", [2, 128, ND * 512], F32R,
                         kind="ExternalInput").ap()
    bT = nc.dram_tensor("bT", [128, 24], F32, kind="ExternalInput").ap()
    bvb = nc.dram_tensor("bvb", [128, D], F32, kind="ExternalInput").ap()
    out = nc.dram_tensor("out", [S, D], F32, kind="ExternalOutput").ap()
    with tile.TileContext(nc) as tc:
        _body(tc, xT, wqk2, wv2, bT, bvb, out, reps=reps)
    nc.compile()
    return nc


_PROGRAM = None


def _get_program():
    global _PROGRAM
    if _PROGRAM is None:
        _PROGRAM = build_program()
    return _PROGRAM


def _prep_inputs(hidden_states, Wq, bq, Wk, bk, Wv, bv):
    hs = np.asarray(hidden_states, dtype=np.float32)
    B = hs.shape[0]
    xT = np.ascontiguousarray(hs.transpose(0, 2, 1))
    wT = np.concatenate(
        [np.asarray(Wq, dtype=np.float32).T,
         np.asarray(Wk, dtype=np.float32).T,
         np.asarray(Wv, dtype=np.float32).T], axis=1)
    # wqk2[hp, p, (proj*8+d)*128 + c] = wT[d*128+p, proj*D + hp*128 + c]
    w4 = wT[:, :2 * D].reshape(ND, 128, 2, NHP, 128)  # [d, p, proj, hp, c]
    wqk2 = np.ascontiguousarray(
        w4.transpose(3, 1, 2, 0, 4).reshape(NHP, 128, 16 * 128))
    # wv2[half, p, d*512 + c] = wT[d*128+p, 2D + half*512 + c]
    wv4 = wT[:, 2 * D:].reshape(ND, 128, 2, 512)      # [d, p, half, c]
    wv2 = np.ascontiguousarray(
        wv4.transpose(2, 1, 0, 3).reshape(2, 128, ND * 512))
    b_all = np.concatenate([np.asarray(bq, dtype=np.float32),
                            np.asarray(bk, dtype=np.float32),
                            np.asarray(bv, dtype=np.float32)])
    bT_np = np.ascontiguousarray(b_all.reshape(24, 128).T)
    bvb_np = np.ascontiguousarray(
        np.broadcast_to(np.asarray(bv, dtype=np.float32), (128, D)))
    return [{"xT": xT[b], "wqk2": wqk2, "wv2": wv2, "bT": bT_np,
             "bvb": bvb_np} for b in range(B)]


def run(in_maps, **kw):
    nc = _get_program()
    return bass_utils.run_bass_kernel_spmd(
        nc, in_maps, core_ids=list(range(len(in_maps))), **kw)


def kernel(hidden_states, Wq, bq, Wk, bk, Wv, bv):
    in_maps = _prep_inputs(hidden_states, Wq, bq, Wk, bk, Wv, bv)
    res = run(in_maps)
    return np.stack([res.results[b]["out"] for b in range(len(in_maps))],
                    axis=0)


# revision 21
# speedup vs baseline: 1.0648x; 1.0570x over previous
"""DINOv2 self-attention (QKV projection + SDPA, no out-proj) on 8 Trainium2
NeuronCores.

Sharding: pure data-parallel over batch (B=8 -> one batch element per core);
no cross-core communication.

Host-side prep inside kernel(): transpose hidden_states to x.T per batch and
pack W as W.T = [Wq.T | Wk.T | Wv.T], so every on-chip matmul operand already
has its contraction dim on the partition axis.

Per-core program (S=1370, D=1024, H=16, hd=64), all matmuls in float32r:
  Phase A: v = x @ Wv^T + bv scattered into v_ext with a ones-column per head
     (v_ext[:, t, h*65+64] = 1) so the softmax denominator falls out of the
     ctx matmul as a 65th output row.  Then q/k projection for head-pair 0.
  Phase B (head-pair pipeline): for hp in 0..7, attention for heads
     2hp,2hp+1 over all of S (sq chunks of <=512), while the q/k projections
     for head-pair hp+1 are drip-fed into the same PE instruction stream
     (2 matmuls per kt block) and the previous chunk's ctx.T finalization
     (PE transpose + DVE normalize, 1 item per kt block) rides along too.
     This keeps ACT's exp stream (the second-busiest engine) running from
     ~45us onward instead of idling during a monolithic projection phase.
  scoresT[sk, sq] = kT^T @ qT per head at partition offsets 0/64 (row-group
     pairs), exp via ACT with fused 1/8 scale (max-subtraction skipped:
     |scores/8| <= ~8.7 fits fp32 easily), ctx.T[65, sq] accumulated over the
     11 sk tiles with [v | 1] stationary.
  Flush: ctx.T transposed back per 128-wide sub-tile via PE transpose, then
     out[:, hp*128:+128] = ctx * (1/denom) on DVE, column-block DMA to out.

PSUM budget (8 banks): scores double-buffer 2x[128,2,512] = 4, ctx
accumulator [65,2,512] = 2, projection pass [128,512] = 1, transpose
scratch [128,65] = 1.
"""

import numpy as np
from contextlib import ExitStack

import concourse.bass as bass
import concourse.bacc as bacc
import concourse.tile as tile
from concourse import mybir
from concourse import bass_utils
from concourse.masks import make_identity

S, D, H, HD = 1370, 1024, 16, 64
F32 = mybir.dt.float32
F32R = mybir.dt.float32r
BF16 = mybir.dt.bfloat16
ND = D // 128                      # 8 contraction tiles
NT = (S + 127) // 128              # 11 token tiles
TSZ = [min(128, S - i * 128) for i in range(NT)]
CHUNKS = [(0, 512), (512, 512), (1024, S - 1024)]
EXP = mybir.ActivationFunctionType.Exp
NHP = 8                            # head pairs
# ablation knobs (diagnostics only; breaks correctness when not default)
ABL_NO_NORM = False     # skip reciprocal+mul (copy raw ctx.T)
ABL_NO_FLUSH = False    # skip transposes too; DMA cst directly
ABL_NO_CTX = False      # skip ctx matmuls (and flush is fed garbage psc)
ABL_NO_EXP = False      # skip exp (et stays garbage)
ABL_NO_SCORES = False   # scores only for kt==0
ABL_NO_OUT = False      # skip output DMAs
EXP_SPLIT = 1           # exp instructions per (hp, kt): 1 or 2


class _Pump:
    """Wraps a generator; pump(n) advances up to n yields, returns number
    actually advanced (0 once exhausted)."""

    def __init__(self, gen):
        self.gen = gen
        self.done = gen is None

    def pump(self, n):
        if self.done:
            return 0
        c = 0
        try:
            while c < n:
                next(self.gen)
                c += 1
        except StopIteration:
            self.done = True
        return c

    def drain(self):
        while not self.done:
            self.pump(64)


def _one_pass(tc, P, xT, wqk2, wv2, out):
    nc = tc.nc
    ident, bT_sb = P.ident, P.bT_sb
    xt = P.xt_pool.tile([128, ND, S], F32R, tag="xt", name="xt")
    v_ext = P.vext_pool.tile([128, NT, H * 65], F32R, tag="vext", name="v_ext")

    # ones columns (h*65+64) for the fused softmax denominator; memset
    # can't produce f32r, so synthesize 1.0 as in0*0 + 1 on DVE
    for t in range(NT):
        ones_view = v_ext[:, t, :].rearrange("p (h e) -> p h e", e=65)[:, :, 64]
        nc.vector.tensor_scalar(
            ones_view, bT_sb[:, 0:16],
            0.0, 1.0, mybir.AluOpType.mult, mybir.AluOpType.add)

    qk_tiles = {}

    def proj_gen(hpn):
        # q/k projection for head-pair hpn: 2 proj x 3 chunks x 8 d-tile
        # matmuls into a single psqk bank, bias added on DVE into qT/kT.
        # All 16 weight tiles arrive in ONE host-packed DMA (DVE queue).
        qTn = P.qk_pool.tile([128, S], F32R, tag="qT", name="qT")
        kTn = P.qk_pool.tile([128, S], F32R, tag="kT", name="kT")
        qk_tiles[hpn] = (qTn, kTn)
        w = P.wqk_pool.tile([128, 16, 128], F32R, tag="wqk", name="wqk")
        nc.sync.dma_start(w[:], wqk2[hpn])
        for proj in (1, 0):
            dstT = kTn if proj == 1 else qTn
            for (c0, cw) in CHUNKS:
                ps = P.psqk.tile([128, 512], F32, tag="psqk", name="psqk")
                for d in range(ND):
                    nc.tensor.matmul(
                        ps[:, :cw], w[:, proj * 8 + d, :], xt[:, d, c0:c0 + cw],
                        start=(d == 0), stop=(d == ND - 1))
                    yield
                nc.vector.tensor_scalar_add(
                    dstT[:, c0:c0 + cw], ps[:, :cw],
                    bT_sb[:, proj * 8 + hpn:proj * 8 + hpn + 1])
                yield

    wv_pool = P.wv_pool
    bvb_sb = wv_pool.tile([128, D], F32, tag="bvb", name="bvb_sb")

    def v_gen(half):
        # v projection for heads half*8..half*8+7, scattered into v_ext;
        # shares the psqk PSUM bank with the q/k projection stream.
        wv = wv_pool.tile([128, ND, 512], F32R, tag="wv", name="wv")
        nc.scalar.dma_start(wv[:], wv2[half])
        for t in range(NT):
            tsz = TSZ[t]
            ps = P.psqk.tile([128, 512], F32, tag="psqk", name="psv")
            for d in range(ND):
                nc.tensor.matmul(
                    ps[:tsz, :], xt[:, d, t * 128:t * 128 + tsz],
                    wv[:, d, :], start=(d == 0), stop=(d == ND - 1))
                yield
            dst = v_ext[:tsz, t, :].rearrange(
                "p (h e) -> p h e", e=65)[:, half * 8:(half + 1) * 8, 0:64]
            src = ps[:tsz, :].rearrange("p (h e) -> p h e", e=64)
            bias = bvb_sb[:tsz, half * 512:(half + 1) * 512].rearrange(
                "p (h e) -> p h e", e=64)
            nc.vector.tensor_add(dst, src, bias)
            yield

    # ---- phase A: x loads, v half 0, q/k projection for hp 0 ----
    for d in range(ND):
        nc.sync.dma_start(xt[:, d, :], xT[d * 128:(d + 1) * 128, :])
    nc.scalar.dma_start(bvb_sb[:], P.bvb[:])
    _Pump(v_gen(0)).drain()
    _Pump(proj_gen(0)).drain()
    _Pump(v_gen(1)).drain()
    vrest = _Pump(None)

    # ---- phase B: per-head-pair attention with pipelined projections ----
    with ExitStack() as sB:
        pss = sB.enter_context(tc.tile_pool(name="pss", bufs=2, space="PSUM"))
        psc = sB.enter_context(tc.tile_pool(name="psc", bufs=1, space="PSUM"))
        tpp = sB.enter_context(tc.tile_pool(name="tpp", bufs=1, space="PSUM"))
        et_pool = sB.enter_context(tc.tile_pool(name="et", bufs=3))
        cs_pool = sB.enter_context(tc.tile_pool(name="cs", bufs=2))
        outp = sB.enter_context(tc.tile_pool(name="outp", bufs=3))
        rec_pool = sB.enter_context(tc.tile_pool(name="rec", bufs=4))

        def flush_gen(cst, hp, c0, cw):
            # finalize a completed (hp, chunk): per sub-tile PE transpose of
            # ctx.T, normalize by the denominator row on DVE, then ONE
            # strided DMA for the whole 128-wide output column block.
            subs = [(s0, min(128, cw - s0)) for s0 in range(0, cw, 128)]
            ot = outp.tile([128, 4, 128], F32, tag="out", name="out_sb")
            if ABL_NO_FLUSH:
                for si, (s0, ssz) in enumerate(subs):
                    nc.vector.tensor_copy(ot[:65, si, :], cst[:65, 0, s0:s0 + 128])
                    yield
            else:
              for si, (s0, ssz) in enumerate(subs):
                for hi in range(2):
                    tp = tpp.tile([128, 65], F32, tag="tp", name="tp")
                    nc.tensor.transpose(
                        tp[:ssz, :], cst[:65, hi, s0:s0 + ssz], ident[:65, :65])
                    if ABL_NO_NORM:
                        nc.vector.tensor_copy(
                            ot[:ssz, si, hi * 64:(hi + 1) * 64], tp[:ssz, 0:64])
                        yield
                        continue
                    rec = rec_pool.tile([128, 1], F32, tag="rec", name="rec")
                    nc.vector.reciprocal(rec[:ssz], tp[:ssz, 64:65])
                    nc.vector.tensor_scalar_mul(
                        ot[:ssz, si, hi * 64:(hi + 1) * 64], tp[:ssz, 0:64],
                        rec[:ssz])
                    yield
            if ABL_NO_OUT:
                return
            full, rem = cw // 128, cw % 128
            cols = slice(hp * 128, (hp + 1) * 128)
            nc.sync.dma_start(
                out[c0:c0 + full * 128, cols].rearrange(
                    "(sub p) c -> p sub c", p=128),
                ot[:, 0:full, :])
            if rem:
                nc.sync.dma_start(out[c0 + full * 128:c0 + cw, cols],
                                  ot[:rem, full, :])

        flush = _Pump(None)
        proj = _Pump(None)
        # v half 1 (heads 8-15) drips into hp 0-1 attention; needed by hp 4
        for hp in range(NHP):
            qT, kT = qk_tiles[hp]
            if hp + 1 < NHP:
                proj = _Pump(proj_gen(hp + 1))
            for (c0, cw) in CHUNKS:
                psc_t = psc.tile([65, 2, 512], F32, tag="psc", name="psc")
                ets = {}

                def emit_ctx(kt):
                    ksz = TSZ[kt]
                    et = ets.pop(kt)
                    if ABL_NO_CTX and kt not in (0, NT - 1):
                        return
                    for hi in range(2):
                        h = 2 * hp + hi
                        nc.tensor.matmul(
                            psc_t[:, hi, :cw],
                            v_ext[:ksz, kt, h * 65:(h + 1) * 65],
                            et[:ksz, hi, :cw],
                            start=(kt == 0), stop=(kt == NT - 1))

                for kt in range(NT):
                    ksz, k0 = TSZ[kt], kt * 128
                    ps_s = pss.tile([128, 2, 512], F32, tag="pss", name="pss")
                    et = et_pool.tile([128, 2, 512], F32R, tag="et", name="et")
                    ets[kt] = et
                    if not (ABL_NO_SCORES and kt > 0):
                        for hi in range(2):
                            p0 = hi * 64
                            nc.tensor.matmul(
                                ps_s[:ksz, hi, :cw],
                                kT[p0:p0 + 64, k0:k0 + ksz],
                                qT[p0:p0 + 64, c0:c0 + cw],
                                start=True, stop=True)
                    if not ABL_NO_EXP:
                        if EXP_SPLIT == 1:
                            nc.scalar.activation(
                                et[:ksz, :, :cw], ps_s[:ksz, :, :cw], EXP,
                                scale=0.125)
                        else:
                            h2 = cw // 2
                            nc.scalar.activation(
                                et[:ksz, :, 0:h2], ps_s[:ksz, :, 0:h2], EXP,
                                scale=0.125)
                            nc.scalar.activation(
                                et[:ksz, :, h2:cw], ps_s[:ksz, :, h2:cw], EXP,
                                scale=0.125)
                    else:
                        nc.vector.tensor_copy(
                            et[:ksz, :, :cw], ps_s[:ksz, :, :cw])
                    if kt > 0:
                        emit_ctx(kt - 1)
                    flush.pump(1)
                    proj.pump(2 if kt % 3 else 1)
                    vrest.pump(0)
                emit_ctx(NT - 1)
                flush.drain()
                cst = cs_pool.tile([65, 2, 512], F32, tag="cs", name="cs")
                nc.vector.tensor_copy(cst[:, :, :cw], psc_t[:, :, :cw])
                flush = _Pump(flush_gen(cst, hp, c0, cw))
        flush.drain()
        proj.drain()
        vrest.drain()


class _Pools:
    pass


def _body(tc, xT, wqk2, wv2, bT, bvb, out, reps=1):
    nc = tc.nc
    with ExitStack() as ctx:
        P = _Pools()
        const = ctx.enter_context(tc.tile_pool(name="const", bufs=1))
        P.ident = const.tile([65, 65], F32)
        make_identity(nc, P.ident)
        P.bT_sb = const.tile([128, 24], F32)
        nc.sync.dma_start(P.bT_sb[:], bT[:])
        P.bvb = bvb
        P.xt_pool = ctx.enter_context(tc.tile_pool(name="xt", bufs=1))
        P.wv_pool = ctx.enter_context(tc.tile_pool(name="wv", bufs=1))
        P.vext_pool = ctx.enter_context(tc.tile_pool(name="vext", bufs=1))
        P.qk_pool = ctx.enter_context(tc.tile_pool(name="qkT", bufs=2))
        P.wqk_pool = ctx.enter_context(tc.tile_pool(name="wqk", bufs=2))
        P.psqk = ctx.enter_context(
            tc.tile_pool(name="psqk", bufs=1, space="PSUM"))
        for _rep in range(reps):
            _one_pass(tc, P, xT, wqk2, wv2, out)


def build_program(reps=1):
    nc = bacc.Bacc("TRN2", target_bir_lowering=False, debug=False,
                   num_devices=8)
    xT = nc.dram_tensor("xT", [D, S], F32R, kind="ExternalInput").ap()
    wqk2 = nc.dram_tensor("wqk2", [NHP, 128, 16 * 128], F32R,
                          kind="ExternalInput").ap()
    wv2 = nc.dram_tensor("wv2", [2, 128, ND * 512], F32R,
                         kind="ExternalInput").ap()
    bT = nc.dram_tensor("bT", [128, 24], F32, kind="ExternalInput").ap()
    bvb = nc.dram_tensor("bvb", [128, D], F32, kind="ExternalInput").ap()
    out = nc.dram_tensor("out", [S, D], F32, kind="ExternalOutput").ap()
    with tile.TileContext(nc) as tc:
        _body(tc, xT, wqk2, wv2, bT, bvb, out, reps=reps)
    nc.compile()
    return nc


_PROGRAM = None


def _get_program():
    global _PROGRAM
    if _PROGRAM is None:
        _PROGRAM = build_program()
    return _PROGRAM


def _prep_inputs(hidden_states, Wq, bq, Wk, bk, Wv, bv):
    hs = np.asarray(hidden_states, dtype=np.float32)
    B = hs.shape[0]
    xT = np.ascontiguousarray(hs.transpose(0, 2, 1))
    wT = np.concatenate(
        [np.asarray(Wq, dtype=np.float32).T,
         np.asarray(Wk, dtype=np.float32).T,
         np.asarray(Wv, dtype=np.float32).T], axis=1)
    # wqk2[hp, p, (proj*8+d)*128 + c] = wT[d*128+p, proj*D + hp*128 + c]
    w4 = wT[:, :2 * D].reshape(ND, 128, 2, NHP, 128)  # [d, p, proj, hp, c]
    wqk2 = np.ascontiguousarray(
        w4.transpose(3, 1, 2, 0, 4).reshape(NHP, 128, 16 * 128))
    # wv2[half, p, d*512 + c] = wT[d*128+p, 2D + half*512 + c]
    wv4 = wT[:, 2 * D:].reshape(ND, 128, 2, 512)      # [d, p, half, c]
    wv2 = np.ascontiguousarray(
        wv4.transpose(2, 1, 0, 3).reshape(2, 128, ND * 512))
    b_all = np.concatenate([np.asarray(bq, dtype=np.float32),
                            np.asarray(bk, dtype=np.float32),
                            np.asarray(bv, dtype=np.float32)])
    bT_np = np.ascontiguousarray(b_all.reshape(24, 128).T)
    bvb_np = np.ascontiguousarray(
        np.broadcast_to(np.asarray(bv, dtype=np.float32), (128, D)))
    return [{"xT": xT[b], "wqk2": wqk2, "wv2": wv2, "bT": bT_np,
             "bvb": bvb_np} for b in range(B)]


def run(in_maps, **kw):
    nc = _get_program()
    return bass_utils.run_bass_kernel_spmd(
        nc, in_maps, core_ids=list(range(len(in_maps))), **kw)


def kernel(hidden_states, Wq, bq, Wk, bk, Wv, bv):
    in_maps = _prep_inputs(hidden_states, Wq, bq, Wk, bk, Wv, bv)
    res = run(in_maps)
    return np.stack([res.results[b]["out"] for b in range(len(in_maps))],
                    axis=0)
